# revision 3
# baseline (speedup 1.0000x reference)
"""Causal self-attention on 8 Trainium2 NeuronCores.

Sharding: core c handles batch b = c//2 and head-group g = c%2 (8 of 16
heads). Per core: qkv projection for its head slice, causal attention
(exp softmax without max-subtraction; probabilities/V in bf16),
per-q-tile pairwise AllGather of the attention output y^T between the
two cores of a batch, then c_proj with output columns sharded by group.
Host only slices inputs / concatenates outputs.

Performance structure (engine-level software pipelining):
- Attention inner loop is lag-1 pipelined: the scores for block k+1 are
  emitted before the PV matmuls of block k, so the PE never waits on
  the Scalar-engine exp; one [128,2,512] exp per k-block.
- Softmax normalization: denominator rows are copied to a [1,1024] SBUF
  row, DMA-reshaped across 16 partitions, one batched DVE reciprocal
  ([16,64], free-size-bound: ~0.5us instead of 8x3.3us), DMA'd back,
  and broadcast with a K=1 matmul whose emission is deferred a few
  blocks so the PE has cover for the chain latency.
- Only x-quarter 0 of the qkv projection runs as a packed pre-phase;
  quarters 1-3 and c_proj(qt) are split into 8-matmul groups and
  emitted as dense PE filler at head-pair boundaries of the attention
  loop. This keeps the tensor-engine MAC duty high enough that the HAM
  clock gate stays at K=8 (2.4 GHz) instead of oscillating to K=4.
- c_proj trails its AllGather by two q-tiles so the collective latency
  is fully hidden; the last no-dependency c_proj tile covers AG(3).
"""

import numpy as np

B, T, C, H = 4, 2048, 1024, 16
D = C // H            # 64
NCORES = 8
GROUPS = [[0, 1], [2, 3], [4, 5], [6, 7]]
QT = 512              # q-tile width (matmul moving dim)
KB = 128              # k-block size (PSUM partition dim)
NQT = T // QT         # 4
HPAIRS = 4            # head pairs per core (8 heads)

_CACHE = {}


# --------------------------------------------------------------------------
# walrus workaround: this toolchain allows only ONE sync-wait per
# instruction. Split the end-of-kernel drain, and hoist excess waits from
# any instruction onto NoOps inserted just before it (same engine).
# --------------------------------------------------------------------------
def _patched_tc_class():
    import concourse.tile as tile
    from concourse.vector_clock import ScopedClock, VectorClock

    class PatchedTileContext(tile.TileContext):
        def _drain_and_barrier(self, tick_clock, wait_clock):
            gc = tick_clock.global_clock
            n = len(gc)
            ahead = [p for p in range(n) if gc[p] > 0]
            for p in ahead:
                vec = [gc[q] if q == p else 0 for q in range(n)]
                inst = self.nc.sync.drain()
                wait_clock.add_sem_waits(
                    inst.ins, ScopedClock({None: VectorClock(vec)})
                )
            if not ahead:
                inst = self.nc.sync.drain()
                wait_clock.add_sem_waits(
                    inst.ins, ScopedClock({None: tick_clock.global_clock})
                )
            self.nc.all_engine_barrier()
            assert self.sems is not None
            popped = self.nc._tile_sem_poison_stack.pop()
            assert popped is self._sem_poison
            self.nc.clear_and_free_semaphores(list(self.sems.allocated().values()))
            self.nc.all_engine_barrier()

    return PatchedTileContext


def _split_sync_waits(nc, max_waits=1):
    import concourse.mybir as mybir

    k = 0
    for f in nc.m.functions:
        for bb in f.blocks:
            newl = []
            dirty = False
            for inst in bb.instructions:
                si = inst.sync_info
                if si is not None and len(si.on_wait) > max_waits:
                    waits = list(si.on_wait)
                    excess, keep = waits[:-max_waits], waits[-max_waits:]
                    for w in excess:
                        k += 1
                        nop = mybir.InstNoOp(
                            name=f"I-waitsplit-{k}", ins=[], outs=[]
                        )
                        nop.engine = inst.engine
                        nop.sync_info = mybir.SyncInfo(on_wait=[w], on_update=[])
                        newl.append(nop)
                    inst.sync_info = mybir.SyncInfo(
                        on_wait=keep, on_update=si.on_update
                    )
                    dirty = True
                newl.append(inst)
            if dirty:
                bb.instructions = newl
    return k


# --------------------------------------------------------------------------
# the Bass program (identical on all 8 cores; only input data differs)
# --------------------------------------------------------------------------
def _build_nc(split_waits=True):
    import concourse.bass as bass
    import concourse.mybir as mybir

    F32 = mybir.dt.float32
    F32R = mybir.dt.float32r
    BF16 = mybir.dt.bfloat16
    EXP = mybir.ActivationFunctionType.Exp
    MULT = mybir.AluOpType.mult
    ADD = mybir.AluOpType.add

    PatchedTileContext = _patched_tc_class()

    nc = bass.Bass()

    # ---- parameters --------------------------------------------------
    xT_p = nc.declare_dram_parameter("xT", [C, T], F32R, isOutput=False)
    wqk_p = nc.declare_dram_parameter("wqk", [C, 1024], F32R, isOutput=False)
    wv_p = nc.declare_dram_parameter("wv", [C, 512], F32R, isOutput=False)
    wp_p = nc.declare_dram_parameter("wp", [C, 512], F32R, isOutput=False)
    bqk_p = nc.declare_dram_parameter("bqk", [128, 8], F32, isOutput=False)
    bv_p = nc.declare_dram_parameter("bv", [1, 512], F32R, isOutput=False)
    bp_p = nc.declare_dram_parameter("bp", [1, 512], F32R, isOutput=False)
    mask_p = nc.declare_dram_parameter("masks", [128, 128], BF16, isOutput=False)
    out_p = nc.declare_dram_parameter("out", [T, 512], F32, isOutput=True)

    with PatchedTileContext(nc) as tc:
        dram_cm = tc.tile_pool(name="dramp", bufs=1, space="DRAM")
        dram = dram_cm.__enter__()
        # per-q-tile internal DRAM for the pairwise allgather of y^T
        y_own = [
            dram.tile([512, QT], BF16, name=f"y_own{qt}", tag=f"y_own{qt}")
            for qt in range(NQT)
        ]
        y_all = [
            dram.tile([1024, QT], BF16, name=f"y_all{qt}", tag=f"y_all{qt}")
            for qt in range(NQT)
        ]

        persist_cm = tc.tile_pool(name="persist", bufs=1)
        persist = persist_cm.__enter__()
        qv_cm = tc.tile_pool(name="qv", bufs=1)
        qv = qv_cm.__enter__()

        # ---- persistent small tensors -------------------------------
        mask_sb = persist.tile([128, 128], BF16)
        nc.sync.dma_start(mask_sb[:], mask_p[:])
        bqk_sb = persist.tile([128, 8], F32)
        nc.sync.dma_start(bqk_sb[:], bqk_p[:])
        bv_sb = persist.tile([1, 512], F32R)
        nc.sync.dma_start(bv_sb[:], bv_p[:])
        bp_sb = persist.tile([1, 512], F32R)
        nc.sync.dma_start(bp_sb[:], bp_p[:])
        ones_row = persist.tile([1, 128], F32R)
        nc.vector.memset(ones_row[:].bitcast(F32), 1.0)
        bv_b = persist.tile([128, 512], F32R)   # bv broadcast to 128 partitions
        bp_b = persist.tile([128, 512], F32R)   # bp broadcast

        # ---- persistent activations ---------------------------------
        # qk_sb[ft]: feature-tile ft of [Q^T | K^T], [128, T]; ft 0..3 = Q
        # (head pair ft), ft 4..7 = K. fp32r.
        qk_sb = [qv.tile([128, T], F32R, name=f"qk{ft}", tag=f"qk{ft}") for ft in range(8)]
        # V_sb[tt]: [128, 8, 65] bf16 — T-chunk tt of V per local head + ones
        v_sb = [qv.tile([128, 8, 65], BF16, name=f"v{tt}", tag=f"v{tt}") for tt in range(16)]
        for tt in range(16):
            nc.vector.memset(v_sb[tt][:, :, 64], 1.0)

        # ======= unified compute phase: proj quarters + attention ====
        # quarter 0 of the qkv projection runs as a packed pre-phase;
        # quarters 1-3 and c_proj(qt) are split into 8-matmul "groups"
        # and emitted as dense PE filler at head-pair boundaries of the
        # attention loop, so the PE never starves while the Scalar
        # engine paces the softmax exp (keeps the HAM clock gate at
        # K=8/2.4GHz).
        from collections import deque

        COPY = mybir.ActivationFunctionType.Copy
        proj_cm = tc.tile_pool(name="proj", bufs=1)
        proj = proj_cm.__enter__()
        cpj_cm = tc.tile_pool(name="cpj", bufs=1)
        cpj = cpj_cm.__enter__()
        attn_cm = tc.tile_pool(name="attn", bufs=1)
        attn = attn_cm.__enter__()
        ps_cm = tc.tile_pool(name="ps", bufs=1, space="PSUM")
        psp = ps_cm.__enter__()

        # ---- weight / x loads (interleaved so the first matmul's
        # operands arrive first) --------------------------------------
        wqk_sb = [proj.tile([128, 1024], F32R, name=f"wqk{kc}", tag=f"wqk{kc}") for kc in range(8)]
        wv_sb = [proj.tile([128, 512], F32R, name=f"wv{kc}", tag=f"wv{kc}") for kc in range(8)]
        wp_sb = [cpj.tile([128, 512], BF16, name=f"wp{kc}", tag=f"wp{kc}") for kc in range(8)]

        def load_xq(tq):
            xt = [
                proj.tile([128, QT], F32R, name=f"xt{tq}_{kc}",
                          tag=f"xt{kc}", bufs=2)
                for kc in range(8)
            ]
            for kc in range(8):
                nc.sync.dma_start(
                    xt[kc][:],
                    xT_p[kc * 128 : (kc + 1) * 128, tq * QT : (tq + 1) * QT],
                )
            return xt

        xt_q = [None] * 4
        xt0 = [
            proj.tile([128, QT], F32R, name=f"xt0_{kc}", tag=f"xt{kc}", bufs=2)
            for kc in range(8)
        ]
        for kc in range(8):
            nc.sync.dma_start(wqk_sb[kc][:], wqk_p[kc * 128 : (kc + 1) * 128, :])
            nc.sync.dma_start(xt0[kc][:], xT_p[kc * 128 : (kc + 1) * 128, 0:QT])
        xt_q[0] = xt0
        for kc in range(8):
            nc.sync.dma_start(wv_sb[kc][:], wv_p[kc * 128 : (kc + 1) * 128, :])
        # c_proj weights: cast to bf16 in-flight on the gpsimd sw-DGE
        for kc in range(8):
            nc.gpsimd.dma_start(wp_sb[kc][:], wp_p[kc * 128 : (kc + 1) * 128, :])
        xt_q[1] = load_xq(1)

        # ---- bias broadcasts via K=1 matmul (ones_row.T @ bias_row) --
        bcv = psp.tile([128, 512], F32, tag="S2", bufs=2, name="bcv")
        nc.tensor.matmul(bcv[:], ones_row[:], bv_sb[:], start=True, stop=True)
        nc.scalar.activation(bv_b[:], bcv[:], COPY)
        bcp = psp.tile([128, 512], F32, tag="S2", bufs=2, name="bcp")
        nc.tensor.matmul(bcp[:], ones_row[:], bp_sb[:], start=True, stop=True)
        nc.scalar.activation(bp_b[:], bcp[:], COPY)

        # ---- projection groups (8 accumulating matmuls + bias) ------
        def qk_group(tq, ft):
            xt = xt_q[tq]
            t0 = tq * QT
            ps = psp.tile([128, QT], F32, tag="S2", bufs=2,
                          name=f"qkg{tq}_{ft}")
            for kc in range(8):
                nc.tensor.matmul(
                    ps[:],
                    wqk_sb[kc][:, ft * 128 : (ft + 1) * 128],
                    xt[kc][:],
                    start=(kc == 0),
                    stop=(kc == 7),
                )
            nc.vector.tensor_scalar_add(
                out=qk_sb[ft][:, t0 : t0 + QT],
                in0=ps[:],
                scalar1=bqk_sb[:, ft : ft + 1],
            )

        def v_group(tq, i):
            xt = xt_q[tq]
            tt16 = tq * 4 + i
            ps = psp.tile([128, 512], F32, tag="S2", bufs=2,
                          name=f"vg{tq}_{i}")
            for kc in range(8):
                nc.tensor.matmul(
                    ps[:],
                    xt[kc][:, i * 128 : (i + 1) * 128],
                    wv_sb[kc][:],
                    start=(kc == 0),
                    stop=(kc == 7),
                )
            nc.vector.tensor_tensor(
                out=v_sb[tt16][:, :, 0:64],
                in0=ps[:].rearrange("p (h d) -> p h d", h=8),
                in1=bv_b[:].rearrange("p (h d) -> p h d", h=8),
                op=ADD,
            )

        # quarter 0: packed pre-phase (Q tiles first)
        for ft in range(8):
            qk_group(0, ft)
        for i in range(4):
            v_group(0, i)

        # ---- c_proj pieces ------------------------------------------
        def cproj_dmas(qt):
            # y_all loads for c_proj(qt); issued on the gpsimd software
            # DGE so they don't head-of-line-block the sync DMA queue
            # while waiting on the AllGather.
            ytq = [
                cpj.tile([128, QT], BF16, name=f"ytq{qt}_{kc}",
                         tag=f"ytq{kc}", bufs=1)
                for kc in range(8)
            ]
            for kc in range(8):
                nc.gpsimd.dma_start(
                    ytq[kc][:], y_all[qt][kc * 128 : (kc + 1) * 128, :]
                )
            return ytq

        def cproj_group(qt, tnl, ytq):
            tn = 4 * qt + tnl
            ps = psp.tile([128, 512], F32, tag="S2", bufs=2,
                          name=f"cp{qt}_{tnl}")
            for kc in range(8):
                nc.tensor.matmul(
                    ps[:],
                    ytq[kc][:, tnl * 128 : (tnl + 1) * 128],
                    wp_sb[kc][:],
                    start=(kc == 0),
                    stop=(kc == 7),
                )
            ot = cpj.tile([128, 512], F32, tag="ot", bufs=2)
            nc.vector.tensor_tensor(
                out=ot[:], in0=ps[:], in1=bp_b[:].bitcast(F32), op=ADD
            )
            nc.sync.dma_start(out_p[tn * 128 : (tn + 1) * 128, :], ot[:])

        # deferred softmax-normalize state: emitted pieces of the
        # previous head-pair's normalize, flushed a few k-blocks into
        # the next head-pair so the PE never waits on the recip chain.
        pend_norm = [None]

        def start_norm(qt, hp, ya, yb):
            # chain: denoms -> [1,1024] row -> [16,64] -> recip ->
            # [1,1024] f32r (for matmul moving operand)
            stg = attn.tile([1, 1024], F32R, tag="stg", bufs=1)
            nc.vector.tensor_copy(stg[:, 0:512].bitcast(F32), ya[64:65, :])
            nc.vector.tensor_copy(stg[:, 512:1024].bitcast(F32), yb[64:65, :])
            rg = attn.tile([16, 64], F32R, tag="rg", bufs=1)
            nc.sync.dma_start(
                rg[:], stg[:].rearrange("o (p f) -> o p f", p=16)
            )
            rr16 = attn.tile([16, 64], F32R, tag="rr16", bufs=1)
            with nc.allow_low_precision(reason="softmax recip"):
                nc.vector.reciprocal(rr16[:].bitcast(F32), rg[:].bitcast(F32))
            rr = attn.tile([1, 1024], F32R, tag="rr", bufs=1)
            nc.sync.dma_start(
                rr[:].rearrange("o (p f) -> o p f", p=16), rr16[:]
            )
            pend_norm[0] = (qt, hp, ya, yb, rr)

        def flush_norm():
            if pend_norm[0] is None:
                return
            qt, hp, ya, yb, rr = pend_norm[0]
            pend_norm[0] = None
            yq = attn.tile([128, QT], BF16, tag="yq", bufs=2)
            for half, yy in ((0, ya), (1, yb)):
                bch = psp.tile([64, QT], F32, tag="S2", bufs=2,
                               name=f"bc{qt}_{hp}_{half}")
                nc.tensor.matmul(
                    bch[:], ones_row[:, 0:64],
                    rr[:, half * QT : (half + 1) * QT],
                    start=True, stop=True,
                )
                cch = attn.tile([64, QT], F32R, tag="cc", bufs=2)
                nc.vector.tensor_copy(cch[:], bch[:])
                nc.vector.tensor_tensor(
                    out=yq[half * 64 : (half + 1) * 64, :],
                    in0=yy[0:64, :],
                    in1=cch[:],
                    op=MULT,
                )
            nc.sync.dma_start(
                y_own[qt][hp * 128 : (hp + 1) * 128, :], yq[:]
            )

        # ---- attention loop with filler scheduling ------------------
        fillers = deque()

        def pop_fillers(boundaries_left):
            n = -(-len(fillers) // boundaries_left) if boundaries_left else 0
            for _ in range(min(n, len(fillers))):
                fillers.popleft()()

        for qt in range(NQT):
            q0 = qt * QT
            nkb = 4 * qt + 4
            # enqueue this round's fillers: next x quarter's projection
            # groups, then c_proj(qt-1) (AllGather needs time to land)
            if qt + 1 <= 3:
                for ft in range(8):
                    fillers.append(lambda tq=qt + 1, ft=ft: qk_group(tq, ft))
                for i in range(4):
                    fillers.append(lambda tq=qt + 1, i=i: v_group(tq, i))
            if qt + 2 <= 3:
                xt_q[qt + 2] = load_xq(qt + 2)
            if qt >= 1:
                cp_ytq = cproj_dmas(qt - 1)
                for tnl in range(4):
                    fillers.append(
                        lambda q=qt - 1, t=tnl, y=cp_ytq: cproj_group(q, t, y)
                    )
            # ---- D: attention for this q-tile (sw-pipelined) ----
            for hp in range(HPAIRS):
                if hp >= 1:
                    pop_fillers(4 - hp + 1)
                ya = psp.tile([65, QT], F32, tag="YA", bufs=2,
                              name=f"ya{qt}_{hp}")
                yb = psp.tile([65, QT], F32, tag="YB", bufs=2,
                              name=f"yb{qt}_{hp}")
                pend = None  # (kb, off, p2)
                for kb in range(nkb):
                    m = kb - 4 * qt  # >=0 on diagonal blocks
                    off = 0 if m < 0 else 128 * m
                    if kb == min(4, nkb - 1):
                        flush_norm()
                    s2 = psp.tile([128, 2 * QT], F32, tag="S2", bufs=2,
                                  name=f"s{qt}_{hp}_{kb}")
                    nc.tensor.matmul(
                        s2[:, off:QT],
                        qk_sb[4 + hp][0:64, kb * KB : (kb + 1) * KB],
                        qk_sb[hp][0:64, q0 + off : q0 + QT],
                        start=True,
                        stop=True,
                    )
                    nc.tensor.matmul(
                        s2[:, QT + off : 2 * QT],
                        qk_sb[4 + hp][64:128, kb * KB : (kb + 1) * KB],
                        qk_sb[hp][64:128, q0 + off : q0 + QT],
                        start=True,
                        stop=True,
                    )
                    p2 = attn.tile([128, 2, QT], BF16, tag="P2", bufs=2,
                                   name=f"p{qt}_{hp}_{kb}")
                    s2v = s2[:].rearrange("p (h q) -> p h q", h=2)
                    nc.scalar.activation(
                        p2[:, :, off:QT], s2v[:, :, off:QT], EXP
                    )
                    if m >= 0:  # triangle mask on the diagonal strip
                        nc.vector.tensor_tensor(
                            out=p2[:, :, off : off + 128],
                            in0=p2[:, :, off : off + 128],
                            in1=mask_sb[:].unsqueeze(1).broadcast_to(
                                [128, 2, 128]
                            ),
                            op=MULT,
                        )
                    if pend is not None:
                        pkb, poff, pp2 = pend
                        nc.tensor.matmul(
                            ya[:, poff:QT],
                            v_sb[pkb][:, 2 * hp, :],
                            pp2[:, 0, poff:QT],
                            start=(pkb == 0),
                            stop=False,
                        )
                        nc.tensor.matmul(
                            yb[:, poff:QT],
                            v_sb[pkb][:, 2 * hp + 1, :],
                            pp2[:, 1, poff:QT],
                            start=(pkb == 0),
                            stop=False,
                        )
                    pend = (kb, off, p2)
                # drain last PV
                pkb, poff, pp2 = pend
                nc.tensor.matmul(
                    ya[:, poff:QT],
                    v_sb[pkb][:, 2 * hp, :],
                    pp2[:, 0, poff:QT],
                    start=(pkb == 0),
                    stop=True,
                )
                nc.tensor.matmul(
                    yb[:, poff:QT],
                    v_sb[pkb][:, 2 * hp + 1, :],
                    pp2[:, 1, poff:QT],
                    start=(pkb == 0),
                    stop=True,
                )
                # start the normalize chain (DVE/DMA only); PE pieces
                # are flushed later under filler cover.
                start_norm(qt, hp, ya, yb)
            # ---- qt-end boundary: drain remaining fillers ------------
            pop_fillers(1)
            flush_norm()  # hp3 of this q-tile
            # ---- E: pairwise allgather of this q-tile's y ----
            nc.gpsimd.collective_compute(
                "AllGather",
                mybir.AluOpType.bypass,
                replica_groups=GROUPS,
                ins=[y_own[qt][:].opt()],
                outs=[y_all[qt][:].opt()],
            )
        cp_ytq = cproj_dmas(NQT - 1)
        for tnl in range(4):
            cproj_group(NQT - 1, tnl, cp_ytq)

        ps_cm.__exit__(None, None, None)
        attn_cm.__exit__(None, None, None)
        cpj_cm.__exit__(None, None, None)
        proj_cm.__exit__(None, None, None)
        qv_cm.__exit__(None, None, None)
        persist_cm.__exit__(None, None, None)
        dram_cm.__exit__(None, None, None)

    if split_waits:
        _split_sync_waits(nc)
    return nc


# --------------------------------------------------------------------------
# host side
# --------------------------------------------------------------------------
def _make_masks():
    import ml_dtypes

    i = np.arange(128)[:, None]
    j = np.arange(128)[None, :]
    return (i <= j).astype(ml_dtypes.bfloat16)  # [128, 128] triangle


def _prep_core_inputs(x, w_attn, b_attn, w_proj, b_proj):
    masks = _make_masks()
    in_maps = []
    for c in range(NCORES):
        b, g = divmod(c, 2)
        sl = slice(512 * g, 512 * (g + 1))
        wq = w_attn[:, 0 * C :][:, sl] * 0.125  # fold 1/sqrt(D)
        wk = w_attn[:, C : 2 * C][:, sl]
        bq = b_attn[0 * C :][sl] * 0.125
        bk = b_attn[C : 2 * C][sl]
        wqk = np.concatenate([wq, wk], axis=1)          # [C, 1024]
        bqk = np.concatenate([bq, bk]).reshape(8, 128).T  # [128, 8]
        in_maps.append(
            {
                "xT": np.ascontiguousarray(x[b].T).astype(np.float32),
                "wqk": np.ascontiguousarray(wqk).astype(np.float32),
                "wv": np.ascontiguousarray(w_attn[:, 2 * C :][:, sl]).astype(
                    np.float32
                ),
                "wp": np.ascontiguousarray(w_proj[:, sl]).astype(np.float32),
                "bqk": np.ascontiguousarray(bqk).astype(np.float32),
                "bv": b_attn[2 * C :][sl].reshape(1, 512).astype(np.float32),
                "bp": b_proj[sl].reshape(1, 512).astype(np.float32),
                "masks": masks,
            }
        )
    return in_maps


def _make_compiled(nc):
    """Build a reusable jitted SPMD callable (mirrors
    bass2jax.run_bass_via_pjrt's multi-core branch, but cached so repeat
    calls don't re-trace)."""
    import jax
    import concourse.mybir as mybir
    from jax.experimental.shard_map import shard_map
    from jax.sharding import Mesh, PartitionSpec
    from concourse import bass2jax

    bass2jax.install_neuronx_cc_hook()
    partition_name = (
        nc.partition_id_tensor.name if nc.partition_id_tensor else None
    )
    in_names, out_names, out_avals, zero_shapes = [], [], [], []
    for alloc in nc.m.functions[0].allocations:
        if not isinstance(alloc, mybir.MemoryLocationSet):
            continue
        name = alloc.memorylocations[0].name
        if alloc.kind == "ExternalInput":
            if name != partition_name:
                in_names.append(name)
        elif alloc.kind == "ExternalOutput":
            out_names.append(name)
            shape = tuple(alloc.tensor_shape)
            dtype = mybir.dt.np(alloc.dtype)
            out_avals.append(jax.core.ShapedArray(shape, dtype))
            zero_shapes.append((shape, dtype))
    n_params = len(in_names)
    in_names_full = list(in_names) + list(out_names)
    if partition_name is not None:
        in_names_full.append(partition_name)
    donate = tuple(range(n_params, n_params + len(out_names)))

    def _body(*args):
        operands = list(args)
        if partition_name is not None:
            operands.append(bass2jax.partition_id_tensor())
        outs = bass2jax._bass_exec_p.bind(
            *operands,
            out_avals=tuple(out_avals),
            in_names=tuple(in_names_full),
            out_names=tuple(out_names),
            lowering_input_output_aliases=(),
            sim_require_finite=True,
            sim_require_nnan=True,
            nc=nc,
        )
        return tuple(outs)

    devices = jax.devices()[:NCORES]
    mesh = Mesh(np.asarray(devices), ("core",))
    in_specs = (PartitionSpec("core"),) * (n_params + len(out_names))
    out_specs = (PartitionSpec("core"),) * len(out_names)
    sharded = jax.jit(
        shard_map(
            _body, mesh=mesh, in_specs=in_specs, out_specs=out_specs,
            check_rep=False,
        ),
        donate_argnums=donate,
        keep_unused=True,
    )
    return {
        "sharded": sharded,
        "in_names": in_names,
        "out_names": out_names,
        "out_avals": out_avals,
        "zero_shapes": zero_shapes,
        "mesh": mesh,
    }


def _get_compiled():
    if "compiled" not in _CACHE:
        _CACHE["compiled"] = _make_compiled(_build_nc())
    return _CACHE["compiled"]


def _concat_inputs(cc, in_maps):
    arrs = []
    for name in cc["in_names"]:
        arrs.append(
            np.concatenate([np.asarray(m[name]) for m in in_maps], axis=0)
        )
    return arrs


def _zeros(cc):
    return [
        np.zeros((NCORES * shape[0], *shape[1:]), dtype)
        for shape, dtype in cc["zero_shapes"]
    ]


def run_spmd(in_maps):
    """Returns an object with .results: list of per-core {name: array}."""
    cc = _get_compiled()
    out_arrs = cc["sharded"](*_concat_inputs(cc, in_maps), *_zeros(cc))
    results = []
    for c in range(NCORES):
        d = {}
        for i, name in enumerate(cc["out_names"]):
            shape = cc["out_avals"][i].shape
            d[name] = np.asarray(out_arrs[i]).reshape(NCORES, *shape)[c]
        results.append(d)

    class _R:
        pass

    r = _R()
    r.results = results
    return r


def kernel(x, w_attn, b_attn, w_proj, b_proj):
    x = np.asarray(x, dtype=np.float32)
    w_attn = np.asarray(w_attn, dtype=np.float32)
    b_attn = np.asarray(b_attn, dtype=np.float32)
    w_proj = np.asarray(w_proj, dtype=np.float32)
    b_proj = np.asarray(b_proj, dtype=np.float32)

    in_maps = _prep_core_inputs(x, w_attn, b_attn, w_proj, b_proj)
    res = run_spmd(in_maps)
    out = np.empty((B, T, C), dtype=np.float32)
    for b in range(B):
        out[b, :, 0:512] = res.results[2 * b]["out"]
        out[b, :, 512:1024] = res.results[2 * b + 1]["out"]
    return out
